# revision 45
# baseline (speedup 1.0000x reference)
"""Trainium2 Bass kernel for the plastic Hebbian RNN cell (nn_Network_32358283608359).

Reference computation (H=4096):
    hactiv   = tanh(x @ Wi + bi + hidden @ (w + alpha * hebb))        [1, H]
    hebb_new = (1 - eta) * hebb + eta * outer(hidden, hactiv)         [H, H]
    activout = softmax(hactiv @ Wo + bo)                              [1, 4]
    valueout = hactiv @ Wv + bv                                       [1, 1]

Sharding: the HxH matrices (w, alpha, hebb) are split column-wise across the
8 NeuronCores (512 columns each).  Each core computes its hactiv shard, its
hebb_new shard, and partial head products (hactiv_s @ Wo_s rows / Wv_s rows).
The host concatenates the shards and reduces the tiny [1,5] head partials
(+bias, softmax) — everything O(H^2) runs on-device.
"""

import sys

for _p in ("/opt/trn_rl_repo",):
    if _p not in sys.path:
        sys.path.insert(0, _p)

import numpy as np

import concourse.bass as bass
import concourse.tile as tile
from concourse import bacc, mybir
from concourse.bass_utils import run_bass_kernel_spmd

H = 4096          # hidden size
NCORES = 8
S = H // NCORES   # shard width = 512
P = 128           # partitions
T = H // P        # 32 k-tiles
KT = 4            # k-tiles per DMA chunk (1 MiB chunks)
U = T // KT       # 8 chunks
# (start_tile, n_tiles) per DMA chunk: mostly 4-tile (1 MiB) chunks, with the
# last chunk split small so the post-load matmul tail is short.
CHUNKS = [(0, 4), (4, 4), (8, 4), (12, 4), (16, 4), (20, 4), (24, 4),
          (28, 2), (30, 2)]
NB = 4            # actions
FP32 = mybir.dt.float32
FP32R = mybir.dt.float32r


def build_bass():
    # Bacc (not plain Bass): its finalize() runs move_matmul_waits_to_ldweights
    # + generate_event_semaphores, which split Tile's multi-wait instructions
    # into the single-wait form this neuronxcc build requires.
    nc = bacc.Bacc()

    x_d = nc.dram_tensor("x", [17, 1], FP32, kind="ExternalInput")
    hid_d = nc.dram_tensor("hidden", [1, H], FP32, kind="ExternalInput")
    hidc_d = nc.dram_tensor("hidc", [P, T], FP32, kind="ExternalInput")
    hebb_d = nc.dram_tensor("hebb", [H, S], FP32, kind="ExternalInput")
    wi_d = nc.dram_tensor("wi", [17, S], FP32, kind="ExternalInput")
    bi_d = nc.dram_tensor("bi", [1, S], FP32, kind="ExternalInput")
    w_d = nc.dram_tensor("w", [H, S], FP32, kind="ExternalInput")
    alpha_d = nc.dram_tensor("alpha", [H, S], FP32, kind="ExternalInput")
    eta_d = nc.dram_tensor("eta", [1, 1], FP32, kind="ExternalInput")
    wowv_d = nc.dram_tensor("wowv", [S, NB + 1], FP32, kind="ExternalInput")

    hact_d = nc.dram_tensor("hact", [1, S], FP32, kind="ExternalOutput")
    hebbn_d = nc.dram_tensor("hebb_new", [H, S], FP32, kind="ExternalOutput")
    head_d = nc.dram_tensor("head", [1, NB + 1], FP32, kind="ExternalOutput")

    with tile.TileContext(nc) as tc:
        with (
            tc.tile_pool(name="consts", bufs=1) as consts,
            tc.tile_pool(name="wa", bufs=3) as wa_pool,
            tc.tile_pool(name="hb", bufs=len(CHUNKS)) as hb_pool,
            tc.tile_pool(name="ah", bufs=6) as ah_pool,
            tc.tile_pool(name="outs", bufs=4) as out_pool,
            tc.tile_pool(name="ph", bufs=1, space="PSUM") as ph_pool,
            tc.tile_pool(name="po", bufs=4, space="PSUM") as po_pool,
            tc.tile_pool(name="pj", bufs=1, space="PSUM") as pj_pool,
            tc.tile_pool(name="pt", bufs=1, space="PSUM") as pt_pool,
            tc.tile_pool(name="phd", bufs=1, space="PSUM") as phd_pool,
        ):
            # ---- constants / small inputs ----
            # All const loads ride the ACT HWDGE ring so the SP ring starts
            # streaming the 24 MiB of w/alpha/hebb immediately.
            # hidden in column layout (host-transposed): hidc[p, t] = hidden[t*128+p]
            hidc = consts.tile([P, T], FP32R)
            nc.scalar.dma_start(out=hidc, in_=hidc_d[:, :].bitcast(FP32R))
            # hidden in row layout (stationary for the outer-product matmuls)
            hidr = consts.tile([1, H], FP32)
            nc.scalar.dma_start(out=hidr, in_=hid_d[0:1, :])

            xa = consts.tile([17, 1], FP32)
            nc.scalar.dma_start(out=xa, in_=x_d[:, :])

            wi_sb = consts.tile([17, S], FP32)
            nc.scalar.dma_start(out=wi_sb, in_=wi_d[:, :])
            bi_sb = consts.tile([1, S], FP32)
            nc.scalar.dma_start(out=bi_sb, in_=bi_d[:, :])

            etab = consts.tile([P, 1], FP32)
            eta_ap = eta_d[0:1, 0:1]
            eta_bcast = bass.AP(
                tensor=eta_ap.tensor, offset=eta_ap.offset, ap=[[0, P], [1, 1]]
            )
            nc.scalar.dma_start(out=etab, in_=eta_bcast)
            omes = consts.tile([P, 1], FP32)  # 1 - eta, per partition
            nc.vector.tensor_scalar(
                out=omes, in0=etab, scalar1=-1.0, scalar2=1.0,
                op0=mybir.AluOpType.mult, op1=mybir.AluOpType.add,
            )

            # eta * hidden, precomputed off the critical path: the phase-2
            # outer-product matmuls then consume tanh's output directly.
            hidr_eta = consts.tile([1, H], FP32R)
            nc.vector.tensor_scalar_mul(hidr_eta, hidr, etab[0:1, :])

            wowv_sb = consts.tile([P, S // P, NB + 1], FP32)
            nc.scalar.dma_start(
                out=wowv_sb, in_=wowv_d.rearrange("(a p) n -> p a n", p=P)
            )

            ones11 = consts.tile([1, 1], FP32)
            nc.vector.memset(ones11, 1.0)
            # Warm the tanh LUT on ScalarE before it's on the critical path.
            actwarm = consts.tile([1, 1], FP32)
            nc.scalar.activation(actwarm, ones11, mybir.ActivationFunctionType.Tanh)

            # ---- phase 1: psum_h[1,S] = hidden @ (w + alpha*hebb) + x @ Wi
            psum_h = ph_pool.tile([1, S], FP32)
            # x @ Wi first: keeps the fp32 K=17 matmul off the tail of the
            # accumulation chain (it would otherwise sit between the last
            # load and tanh on the critical path).
            nc.tensor.matmul(psum_h, lhsT=xa, rhs=wi_sb, start=True, stop=False)
            # bias as a rank-1 matmul (ones^T @ bi) so tanh can read PSUM
            # directly with no extra DVE add on the critical path.
            nc.tensor.matmul(psum_h, lhsT=ones11, rhs=bi_sb, start=False, stop=False)
            hebb_chunks = []
            w_chunks = []
            for ci, (t0, csz) in enumerate(CHUNKS):
                rows = slice(t0 * P, (t0 + csz) * P)
                # alpha+hebb first: the ah products compute while w streams in,
                # so the last chunk's tail is just its matmuls.
                a_c = wa_pool.tile([P, csz, S], FP32, tag="a_c")
                nc.sync.dma_start(
                    out=a_c, in_=alpha_d[rows, :].rearrange("(a p) n -> p a n", p=P)
                )
                h_c = hb_pool.tile([P, csz, S], FP32)
                nc.sync.dma_start(
                    out=h_c, in_=hebb_d[rows, :].rearrange("(a p) n -> p a n", p=P)
                )
                w_c = wa_pool.tile([P, csz, S], FP32R, tag="w_c")
                nc.sync.dma_start(
                    out=w_c,
                    in_=w_d[rows, :].rearrange("(a p) n -> p a n", p=P).bitcast(FP32R),
                )
                hebb_chunks.append(h_c)
                w_chunks.append(w_c)
                if ci == len(CHUNKS) - 1:
                    # Warm-up burst while the tail chunks stream in: junk
                    # matmuls gated on the previous chunk's w tile (f32r, so
                    # the fp32r-producer rule holds) keep the PE
                    # pipeline/clock ramped so the final real matmuls run at
                    # full rate instead of paying cold-restart penalties.
                    ps_j = pj_pool.tile([1, S], FP32)
                    for _ in range(4):
                        nc.tensor.matmul(
                            ps_j, lhsT=hidc[:, 0:1], rhs=w_chunks[-2][:, 0, :],
                            start=True, stop=True, skip_group_check=True,
                        )
                for kt in range(csz):
                    t = t0 + kt
                    ah = ah_pool.tile([P, S], FP32R)
                    nc.vector.tensor_mul(ah, a_c[:, kt, :], h_c[:, kt, :])
                    # float32r: 1 PE cycle/row vs 4 for exact fp32; the
                    # matvec feeds tanh, ~1e-6 relative error is fine.
                    nc.tensor.matmul(
                        psum_h, lhsT=hidc[:, t : t + 1], rhs=w_c[:, kt, :],
                        start=False, stop=False,
                    )
                    nc.tensor.matmul(
                        psum_h, lhsT=hidc[:, t : t + 1], rhs=ah,
                        start=False, stop=(t == T - 1),
                    )
            hact_r = consts.tile([1, S], FP32R)
            nc.scalar.activation(
                hact_r, psum_h, mybir.ActivationFunctionType.Tanh
            )
            # Second tanh at full fp32 for the hactiv output and head path
            # (off the critical path; the f32r copy above feeds phase 2).
            hact_row = consts.tile([1, S], FP32)
            nc.scalar.activation(
                hact_row, psum_h, mybir.ActivationFunctionType.Tanh
            )
            nc.sync.dma_start(out=hact_d[:, :], in_=hact_row)

            # ---- phase 2: hebb_new = (1-eta)*hebb + outer(hidden, eta*hactiv)
            for ci, (t0, csz) in enumerate(CHUNKS):
                h_c = hebb_chunks[ci]
                o_c = out_pool.tile([P, csz, S], FP32)
                for kt in range(csz):
                    t = t0 + kt
                    ps_o = po_pool.tile([P, S], FP32)
                    nc.tensor.matmul(
                        ps_o, lhsT=hidr_eta[:, t * P : (t + 1) * P], rhs=hact_r,
                        start=True, stop=True,
                    )
                    nc.vector.scalar_tensor_tensor(
                        out=o_c[:, kt, :], in0=h_c[:, kt, :], scalar=omes,
                        in1=ps_o,
                        op0=mybir.AluOpType.mult, op1=mybir.AluOpType.add,
                    )
                # k-tile-granular stores, alternating between the two HWDGE
                # rings (ACT + SP; SP's load stream is done by now): each
                # store starts as soon as its blend lands.
                for kt in range(csz):
                    t = t0 + kt
                    eng = nc.scalar if t % 2 == 0 else nc.sync
                    eng.dma_start(
                        out=hebbn_d[t * P : (t + 1) * P, :],
                        in_=o_c[:, kt, :],
                    )

            # ---- head partials: [1,5] = hactiv_s @ [Wo_s | Wv_s]
            ps_t = pt_pool.tile([P, S // P], FP32)
            for c in range(S // P):
                nc.tensor.matmul(
                    ps_t[:, c : c + 1], lhsT=hact_row[:, c * P : (c + 1) * P],
                    rhs=ones11, start=True, stop=True,
                )
            hct = consts.tile([P, S // P], FP32)
            nc.vector.tensor_copy(hct, ps_t)
            ps_head = phd_pool.tile([1, NB + 1], FP32)
            for c in range(S // P):
                nc.tensor.matmul(
                    ps_head, lhsT=hct[:, c : c + 1], rhs=wowv_sb[:, c, :],
                    start=(c == 0), stop=(c == S // P - 1),
                )
            head_sb = consts.tile([1, NB + 1], FP32)
            nc.vector.tensor_copy(head_sb, ps_head)
            nc.sync.dma_start(out=head_d[:, :], in_=head_sb)

    # Run the bacc passes (register allocation, event-semaphore splitting for
    # the 1-wait-per-instruction ISA constraint) before serialization.
    nc.finalize()
    return nc


_NC = None


def _get_nc():
    global _NC
    if _NC is None:
        _NC = build_bass()
    return _NC


def make_in_maps(x, hidden, hebb, Wi, bi, w, alpha, eta, Wo, bo, Wv, bv):
    in_maps = []
    for c in range(NCORES):
        J = slice(c * S, (c + 1) * S)
        in_maps.append({
            "x": np.ascontiguousarray(x.reshape(17, 1), dtype=np.float32),
            "hidden": np.ascontiguousarray(hidden, dtype=np.float32),
            "hidc": np.ascontiguousarray(
                hidden.reshape(T, P).T, dtype=np.float32
            ),
            "hebb": np.ascontiguousarray(hebb[:, J], dtype=np.float32),
            "wi": np.ascontiguousarray(Wi[:, J], dtype=np.float32),
            "bi": np.ascontiguousarray(bi[J].reshape(1, S), dtype=np.float32),
            "w": np.ascontiguousarray(w[:, J], dtype=np.float32),
            "alpha": np.ascontiguousarray(alpha[:, J], dtype=np.float32),
            "eta": np.ascontiguousarray(eta.reshape(1, 1), dtype=np.float32),
            "wowv": np.ascontiguousarray(
                np.hstack([Wo[J, :], Wv[J, :]]), dtype=np.float32
            ),
        })
    return in_maps


def assemble_outputs(results, bo, bv):
    hactiv = np.concatenate([r["hact"] for r in results], axis=1)
    hebb_new = np.concatenate([r["hebb_new"] for r in results], axis=1)
    head = np.sum(np.stack([r["head"] for r in results]), axis=0)  # [1,5]
    logits = head[0, :NB] + bo
    m = logits.max()
    e = np.exp(logits - m)
    activout = (e / e.sum()).reshape(1, NB).astype(np.float32)
    valueout = np.array([[head[0, NB] + bv[0]]], dtype=np.float32)
    return (
        activout,
        valueout,
        hactiv.astype(np.float32),
        hebb_new.astype(np.float32),
    )


def run(trace=False, **inputs):
    """Run the SPMD kernel; returns (outputs_tuple, BassKernelResults)."""
    nc = _get_nc()
    in_maps = make_in_maps(**inputs)
    res = run_bass_kernel_spmd(
        nc, in_maps, list(range(NCORES)), trace=trace
    )
    outs = assemble_outputs(res.results, inputs["bo"], inputs["bv"])
    return outs, res


def kernel(**inputs):
    outs, _ = run(trace=False, **inputs)
    return outs
